# revision 28
# baseline (speedup 1.0000x reference)
"""ALCOVE cell Bass kernel for 8 TRN2 NeuronCores (data-parallel over batch).

B=32, T=16, N_RBF=1024, N_DIM=64, UNITS=64. 4 batches per core.

Layout: R=1024 on partitions as 8 chunks of 128; per-batch row data
(att, x, dx, g_att) on partition 0 as (1, B_LOC*64) rows (PE base-partition
rule); partition broadcasts via K=1 ones-matmul. Big elementwise work runs
on (128, B_LOC*NCHUNK*64) = (128, 2048) tiles in single instructions;
contractions over the free dim use TT-mult + tensor_reduce; contractions
over partitions use M=1 accumulating matmuls.
"""

import numpy as np

B, T, R, D, U = 32, 16, 1024, 64, 64
NCHUNK, P = 8, 128
EPS = 1e-6
N_CORES = 8
B_LOC = B // N_CORES  # 4

_cache = {}


def _patch_act_tables():
    """Make every activation resolve to natural_log_exp_and_others (it
    contains abs/ln/exp/relu/copy/identity/square) so the kernel needs a
    single ACT table load instead of thrashing between sets."""
    import concourse.bacc as bacc_mod
    from concourse.hw_specs import get_activation_tables as _gat

    if getattr(bacc_mod.get_activation_tables, "_alcove_patched", False):
        return

    def patched(arch):
        t = _gat(arch)
        keep = t["natural_log_exp_and_others"]
        out = {}
        for name, fns in t.items():
            out[name] = fns if name == "natural_log_exp_and_others" else (fns - keep)
        return out

    patched._alcove_patched = True
    bacc_mod.get_activation_tables = patched


def _build(rho, temperature, lr_att, lr_assoc, beta):
    import concourse.bass as bass
    import concourse.tile as tile
    from concourse import bacc, mybir

    _patch_act_tables()

    f32 = mybir.dt.float32
    bf16 = mybir.dt.bfloat16
    AF = mybir.ActivationFunctionType
    OP = mybir.AluOpType

    nc = bacc.Bacc("TRN2", target_bir_lowering=False, debug=False, num_devices=N_CORES)
    # packed bf16 input: [embedB (512) | zbcast (4096)]; f32 input: [oh (1024) | eye4 (4)]
    FINB = NCHUNK * D + T * B_LOC * D
    FIN = T * B_LOC * U + B_LOC
    bigb_in = nc.declare_dram_parameter("bigb", [P, FINB], bf16, isOutput=False)
    big_in = nc.declare_dram_parameter("big", [P, FIN], f32, isOutput=False)
    out_ext = nc.declare_dram_parameter("out", [B_LOC, T * U], f32, isOutput=True)

    with tile.TileContext(nc) as tc:
        with (
            tc.tile_pool(name="persist", bufs=1) as persist,
            tc.tile_pool(name="work", bufs=3) as work,
            tc.tile_pool(name="psum", bufs=1, space="PSUM") as psum,
            tc.tile_pool(name="psmall", bufs=2, space="PSUM") as psmall,
        ):
            # ---- persistent tiles (one DMA for all inputs) ----
            bigb = persist.tile([P, FINB], bf16)
            nc.gpsimd.dma_start(bigb[:], bigb_in[:])
            big = persist.tile([P, FIN], f32)
            nc.gpsimd.dma_start(big[:], big_in[:])
            embedB = bigb[:, 0 : NCHUNK * D]
            zb = bigb[:, NCHUNK * D :].rearrange("p (t f) -> p t f", t=T)
            # oh rows replicated on all partitions, layout (t, b, u)
            oh = big[0:B_LOC, 0 : T * B_LOC * U].rearrange("p (t b u) -> p t b u", t=T, b=B_LOC)
            eye4 = big[0:B_LOC, T * B_LOC * U :]  # (4, 4) identity
            eye_bc = eye4[:, :, None].broadcast_to([B_LOC, B_LOC, U])

            ones4 = persist.tile([B_LOC, P], bf16)
            nc.vector.memset(ones4[:], 1.0)
            consts = persist.tile([P, 3], f32)
            nc.vector.memset(consts[:, 0:1], 0.0)
            nc.vector.memset(consts[:, 1:2], 1.0)
            nc.vector.memset(consts[:, 2:3], EPS)
            czero, cone, ceps = consts[:, 0:1], consts[:, 1:2], consts[:, 2:3]

            attb = persist.tile([P, B_LOC, D], bf16)  # state, replicated on partitions
            nc.vector.memset(attb[:], 1.0 / D)
            assoc = persist.tile([P, B_LOC, NCHUNK, U], bf16)  # state
            nc.vector.memset(assoc[:], 0.0)

            probs_row = persist.tile([1, T, B_LOC, U], f32)

            # broadcast-view of embedB over batches: (P, B_LOC, NCHUNK, D)
            embed_bc = embedB.rearrange("p (c d) -> p c d", d=D)[:, None, :, :].broadcast_to([P, B_LOC, NCHUNK, D])

            for t in range(T):
                # -------- diff^rho chain on (P, B_LOC, NCHUNK, D) in one shot
                zrep = zb[:, t, :].rearrange("p (b d) -> p b d", d=D)[:, :, None, :].broadcast_to([P, B_LOC, NCHUNK, D])
                diff = work.tile([P, B_LOC, NCHUNK, D], bf16, tag="diff", bufs=2)
                nc.vector.tensor_tensor(diff[:], embed_bc, zrep, op=OP.subtract)
                nc.scalar.activation(diff[:], diff[:], AF.Abs, bias=czero)
                nc.scalar.activation(diff[:], diff[:], AF.Ln, bias=ceps)
                dpow = work.tile([P, B_LOC, NCHUNK, D], bf16, tag="dpow", bufs=2)
                nc.scalar.activation(dpow[:], diff[:], AF.Exp, bias=czero, scale=rho)

                # -------- q = sum_d att*dpow : TT mult + reduce
                qtmp = work.tile([P, B_LOC, NCHUNK, D], bf16, tag="qtmp", bufs=2)
                attb_bc = attb[:, :, None, :].broadcast_to([P, B_LOC, NCHUNK, D])
                nc.vector.tensor_tensor(qtmp[:], dpow[:], attb_bc, op=OP.mult)
                qall = work.tile([P, B_LOC, NCHUNK], f32, tag="qall")
                nc.vector.tensor_reduce(qall[:], qtmp[:], axis=mybir.AxisListType.X, op=OP.add)

                # -------- similarity acts on (P, B_LOC*NCHUNK)
                lnq = work.tile([P, B_LOC, NCHUNK], f32, tag="lnq")
                nc.scalar.activation(lnq[:], qall[:], AF.Ln, bias=ceps)
                dd = work.tile([P, B_LOC, NCHUNK], f32, tag="dd")
                nc.scalar.activation(dd[:], lnq[:], AF.Exp, bias=czero, scale=1.0 / rho)   # = d
                s_sim = work.tile([P, B_LOC, NCHUNK], bf16, tag="s_sim")
                nc.scalar.activation(s_sim[:], dd[:], AF.Exp, bias=czero, scale=-beta)     # = s (bf16)
                qp = work.tile([P, B_LOC, NCHUNK], bf16, tag="qp")
                nc.scalar.activation(qp[:], lnq[:], AF.Exp, bias=czero, scale=(1.0 - rho) / rho)


                # -------- x: M=4 packed matmuls (one per chunk), diag extract
                x_ps = psmall.tile([B_LOC, B_LOC, U], f32, tag="x_ps")
                for c in range(NCHUNK):
                    nc.tensor.matmul(x_ps[:, :, :],
                                     s_sim[:, :, c],
                                     assoc[:, :, c, :],
                                     start=(c == 0), stop=(c == NCHUNK - 1))
                # -------- teacher / dx on the cross tile (diag cols are the real x)
                pp = work.tile([B_LOC, B_LOC, U], f32, tag="pp")
                nc.scalar.activation(pp[:], x_ps[:], AF.Relu, bias=cone[:B_LOC, :])          # relu(x+1)
                mrow = work.tile([B_LOC, B_LOC, U], f32, tag="mrow")
                nc.scalar.activation(mrow[:], x_ps[:], AF.Relu, bias=cone[:B_LOC, :], scale=-1.0)  # relu(1-x)
                nc.vector.tensor_tensor(mrow[:], pp[:], mrow[:], op=OP.add)            # p+m
                nc.vector.tensor_tensor(mrow[:], mrow[:], oh[:, t, :, :], op=OP.mult)  # oh*(p+m)
                dxf = work.tile([B_LOC, B_LOC, U], f32, tag="dxf")
                nc.vector.tensor_tensor(dxf[:], pp[:], mrow[:], op=OP.subtract)
                # mask off-diagonal cross terms (bf16 out for single-pass matmuls)
                dxc = work.tile([B_LOC, B_LOC, U], bf16, tag="dxc")
                nc.vector.tensor_tensor(dxc[:], dxf[:], eye_bc, op=OP.mult)
                xm = work.tile([B_LOC, B_LOC, U], bf16, tag="xm")
                nc.vector.tensor_tensor(xm[:], x_ps[:], eye_bc, op=OP.mult)

                # -------- dx broadcast to (P, B_LOC, U): ones4^T @ dxm
                dxb_ps = psum.tile([P, B_LOC, U], f32, tag="dxb")
                nc.tensor.matmul(dxb_ps[:, :, :].rearrange("p b d -> p (b d)"),
                                 ones4[:], dxc[:].rearrange("p b u -> p (b u)"),
                                 start=True, stop=True)
                # x as a single row (1, B*U): ones4[:, :1]^T @ xm
                xrow_ps = psmall.tile([1, B_LOC, U], f32, tag="rowps")
                nc.tensor.matmul(xrow_ps[:, :, :].rearrange("p b u -> p (b u)"),
                                 ones4[:, 0:1], xm[:].rearrange("p b u -> p (b u)"),
                                 start=True, stop=True)

                # -------- softmax(temp*x) in row layout, per-b stable
                xr = work.tile([1, B_LOC, U], f32, tag="xr")
                nc.scalar.copy(xr[:], xrow_ps[:])
                mx4 = work.tile([1, B_LOC], f32, tag="mx4")
                nc.vector.tensor_reduce(mx4[:], xr[:], axis=mybir.AxisListType.X, op=OP.max)
                mx_bc = mx4[:, :, None].broadcast_to([1, B_LOC, U])
                xs_t = work.tile([1, B_LOC, U], f32, tag="xs_t")
                nc.vector.tensor_tensor(xs_t[:], xr[:], mx_bc, op=OP.subtract)
                er = work.tile([1, B_LOC, U], f32, tag="er")
                nc.scalar.activation(er[:], xs_t[:], AF.Exp, bias=czero[:1, :], scale=temperature)
                sm4 = work.tile([1, B_LOC], f32, tag="sm4")
                nc.vector.tensor_reduce(sm4[:], er[:], axis=mybir.AxisListType.X, op=OP.add)
                rc4 = work.tile([1, B_LOC], f32, tag="rc4")
                nc.vector.reciprocal(rc4[:], sm4[:])
                rc_bc = rc4[:, :, None].broadcast_to([1, B_LOC, U])
                nc.vector.tensor_tensor(probs_row[:, t, :, :], er[:], rc_bc, op=OP.mult)
                dxb = work.tile([P, B_LOC, U], bf16, tag="dxb_sb")
                nc.scalar.copy(dxb[:], dxb_ps[:])
                dxb_bc = dxb[:, :, None, :].broadcast_to([P, B_LOC, NCHUNK, U])

                # -------- y = sum_u assoc*dx : TT mult + reduce
                ytmp = work.tile([P, B_LOC, NCHUNK, U], bf16, tag="ytmp", bufs=2)
                nc.vector.tensor_tensor(ytmp[:], assoc[:], dxb_bc, op=OP.mult)
                yall = work.tile([P, B_LOC, NCHUNK], f32, tag="yall")
                nc.vector.tensor_reduce(yall[:], ytmp[:], axis=mybir.AxisListType.X, op=OP.add)

                # -------- c = -(beta/rho) * s * qp * y
                sqp = work.tile([P, B_LOC, NCHUNK], f32, tag="sqp")
                nc.vector.tensor_tensor(sqp[:], s_sim[:], qp[:], op=OP.mult)
                call_b16 = work.tile([P, B_LOC, NCHUNK], bf16, tag="call_b16")
                nc.vector.scalar_tensor_tensor(call_b16[:], yall[:], -beta / rho, sqp[:],
                                               op0=OP.mult, op1=OP.mult)

                # -------- g_att: M=4 packed matmuls + diag extract + att update
                gatt_ps = psmall.tile([B_LOC, B_LOC, D], f32, tag="gatt")
                for c in range(NCHUNK):
                    nc.tensor.matmul(gatt_ps[:, :, :],
                                     call_b16[:, :, c],
                                     dpow[:, :, c, :],
                                     start=(c == 0), stop=(c == NCHUNK - 1))
                gm = work.tile([B_LOC, B_LOC, D], bf16, tag="gm")
                nc.vector.tensor_tensor(gm[:], gatt_ps[:], eye_bc, op=OP.mult)
                grow_ps = psum.tile([P, B_LOC, D], f32, tag="grow")
                nc.tensor.matmul(grow_ps[:, :, :].rearrange("p b d -> p (b d)"),
                                 ones4[:], gm[:].rearrange("p b d -> p (b d)"),
                                 start=True, stop=True)
                nc.vector.scalar_tensor_tensor(attb[:], grow_ps[:], -lr_att, attb[:],
                                               op0=OP.mult, op1=OP.add)
                nc.vector.tensor_scalar_max(attb[:], attb[:], 0.0)

                # -------- assoc update: assoc += (-lr*s_bc) * dx_bu (2 big TTs)
                slr = work.tile([P, B_LOC, NCHUNK], bf16, tag="slr")
                nc.vector.tensor_scalar_mul(slr[:], s_sim[:], -lr_assoc)
                upd = work.tile([P, B_LOC, NCHUNK, U], bf16, tag="upd", bufs=2)
                slr_bc = slr[:, :, :, None].broadcast_to([P, B_LOC, NCHUNK, U])
                nc.gpsimd.tensor_tensor(upd[:], slr_bc, dxb_bc, op=OP.mult)
                nc.gpsimd.tensor_tensor(assoc[:], assoc[:], upd[:], op=OP.add)

            # -------- store: (1, T, B, U) row -> (B, T*U), one DMA per batch
            for b in range(B_LOC):
                nc.gpsimd.dma_start(out_ext[b : b + 1, :].rearrange("b (t u) -> b t u", t=T),
                                    probs_row[0:1, :, b, :])

    nc.compile()
    return nc


def _prep_in_maps(stimulus_set, label_idx, embed):
    embedB = embed.reshape(NCHUNK, P, D).transpose(1, 0, 2).reshape(P, NCHUNK * D)
    z = embed[stimulus_set]  # (B, T, D)
    onehot = np.zeros((B, T, U), dtype=np.float32)
    bi, ti = np.meshgrid(np.arange(B), np.arange(T), indexing="ij")
    onehot[bi, ti, label_idx] = 1.0
    in_maps = []
    for i in range(N_CORES):
        bs = slice(i * B_LOC, (i + 1) * B_LOC)
        zc = z[bs].transpose(1, 0, 2).reshape(1, T * B_LOC * D)
        zbcast = np.broadcast_to(zc, (P, T * B_LOC * D))
        ohrow = onehot[bs].transpose(1, 0, 2).reshape(1, T * B_LOC * U)
        ohfull = np.broadcast_to(ohrow, (P, T * B_LOC * U))
        eyefull = np.zeros((P, B_LOC), dtype=np.float32)
        eyefull[:B_LOC, :] = np.eye(B_LOC, dtype=np.float32)
        import ml_dtypes
        bigb = np.concatenate([embedB, zbcast], axis=1).astype(ml_dtypes.bfloat16)
        big = np.concatenate([ohfull, eyefull], axis=1).astype(np.float32)
        in_maps.append({"bigb": np.ascontiguousarray(bigb),
                        "big": np.ascontiguousarray(big)})
    return in_maps


def kernel(stimulus_set, label_idx, embed, rho, temperature, lr_attention, lr_association, beta):
    from concourse.bass_utils import run_bass_kernel_spmd

    stimulus_set = np.asarray(stimulus_set)
    label_idx = np.asarray(label_idx)
    embed = np.asarray(embed, dtype=np.float32)
    key = (float(rho), float(temperature), float(lr_attention),
           float(lr_association), float(beta))
    if key not in _cache:
        _cache[key] = _build(*key)
    nc = _cache[key]
    in_maps = _prep_in_maps(stimulus_set, label_idx, embed)
    res = run_bass_kernel_spmd(nc, in_maps, core_ids=list(range(N_CORES)))
    outs = [res.results[i]["out"].reshape(B_LOC, T, U) for i in range(N_CORES)]
    return np.concatenate(outs, axis=0)


def _install_ntff_hook():
    import sys, types, ctypes, contextlib
    if "antenv.axon_hooks" in sys.modules:
        return
    import antenv
    mod = types.ModuleType("antenv.axon_hooks")
    mod._hook = None
    def set_axon_ntff_profile_hook(h):
        mod._hook = h
    def get_axon_ntff_profile_hook():
        return mod._hook
    mod.set_axon_ntff_profile_hook = set_axon_ntff_profile_hook
    mod.get_axon_ntff_profile_hook = get_axon_ntff_profile_hook
    sys.modules["antenv.axon_hooks"] = mod
    antenv.axon_hooks = mod

    lib = ctypes.CDLL("/opt/axon/libaxon_pjrt.so")
    if not hasattr(lib, "axon_start_nrt_profile"):
        return
    lib.axon_start_nrt_profile.argtypes = [ctypes.POINTER(ctypes.c_int64), ctypes.c_size_t]
    lib.axon_start_nrt_profile.restype = ctypes.c_int64
    lib.axon_stop_nrt_profile.argtypes = [ctypes.c_char_p]
    lib.axon_stop_nrt_profile.restype = ctypes.c_int64

    @contextlib.contextmanager
    def _hook(output_dir, device_ids):
        import jax
        jax.devices()
        if device_ids:
            ids = (ctypes.c_int64 * len(device_ids))(*device_ids)
            rc = lib.axon_start_nrt_profile(ids, len(device_ids))
        else:
            rc = lib.axon_start_nrt_profile(None, 0)
        if rc != 0:
            raise RuntimeError(f"axon_start_nrt_profile rc={rc}")
        try:
            yield
        finally:
            n = lib.axon_stop_nrt_profile(str(output_dir).encode())
            print(f"profile: {n} file(s) written to {output_dir}")

    set_axon_ntff_profile_hook(_hook)


def kernel_traced(**inputs):
    """Like kernel() but runs with NTFF tracing; returns (out, exec_time_ns, tmpdir)."""
    import tempfile
    _install_ntff_hook()
    from concourse.bass_utils import run_bass_kernel_spmd

    key = (float(inputs["rho"]), float(inputs["temperature"]), float(inputs["lr_attention"]),
           float(inputs["lr_association"]), float(inputs["beta"]))
    if key not in _cache:
        _cache[key] = _build(*key)
    nc = _cache[key]
    in_maps = _prep_in_maps(np.asarray(inputs["stimulus_set"]), np.asarray(inputs["label_idx"]),
                            np.asarray(inputs["embed"], dtype=np.float32))
    tmpdir = tempfile.mkdtemp(prefix="alcove_trace_")
    res = run_bass_kernel_spmd(nc, in_maps, core_ids=list(range(N_CORES)), trace=True, tmpdir=tmpdir)
    outs = [res.results[i]["out"].reshape(B_LOC, T, U) for i in range(N_CORES)]
    return np.concatenate(outs, axis=0), res.exec_time_ns, tmpdir


# revision 29
# speedup vs baseline: 1.0977x; 1.0977x over previous
"""ALCOVE cell Bass kernel for 8 TRN2 NeuronCores (data-parallel over batch).

B=32, T=16, N_RBF=1024, N_DIM=64, UNITS=64. 4 batches per core.

Layout: R=1024 on partitions as 8 chunks of 128; per-batch row data
(att, x, dx, g_att) on partition 0 as (1, B_LOC*64) rows (PE base-partition
rule); partition broadcasts via K=1 ones-matmul. Big elementwise work runs
on (128, B_LOC*NCHUNK*64) = (128, 2048) tiles in single instructions;
contractions over the free dim use TT-mult + tensor_reduce; contractions
over partitions use M=1 accumulating matmuls.
"""

import numpy as np

B, T, R, D, U = 32, 16, 1024, 64, 64
NCHUNK, P = 8, 128
EPS = 1e-6
N_CORES = 8
B_LOC = B // N_CORES  # 4

_cache = {}


def _patch_act_tables():
    """Make every activation resolve to natural_log_exp_and_others (it
    contains abs/ln/exp/relu/copy/identity/square) so the kernel needs a
    single ACT table load instead of thrashing between sets."""
    import concourse.bacc as bacc_mod
    from concourse.hw_specs import get_activation_tables as _gat

    if getattr(bacc_mod.get_activation_tables, "_alcove_patched", False):
        return

    def patched(arch):
        t = _gat(arch)
        keep = t["natural_log_exp_and_others"]
        out = {}
        for name, fns in t.items():
            out[name] = fns if name == "natural_log_exp_and_others" else (fns - keep)
        return out

    patched._alcove_patched = True
    bacc_mod.get_activation_tables = patched


def _build(rho, temperature, lr_att, lr_assoc, beta):
    import concourse.bass as bass
    import concourse.tile as tile
    from concourse import bacc, mybir

    _patch_act_tables()

    f32 = mybir.dt.float32
    bf16 = mybir.dt.bfloat16
    AF = mybir.ActivationFunctionType
    OP = mybir.AluOpType

    nc = bacc.Bacc("TRN2", target_bir_lowering=False, debug=False, num_devices=N_CORES)
    # packed bf16 input: [embedB (512) | zbcast (4096)]; f32 input: [oh (1024) | eye4 (4)]
    FINB = NCHUNK * D + T * B_LOC * D
    FIN = T * B_LOC * U + B_LOC
    bigb_in = nc.declare_dram_parameter("bigb", [P, FINB], bf16, isOutput=False)
    big_in = nc.declare_dram_parameter("big", [P, FIN], f32, isOutput=False)
    out_ext = nc.declare_dram_parameter("out", [B_LOC, T * U], f32, isOutput=True)

    with tile.TileContext(nc) as tc:
        with (
            tc.tile_pool(name="persist", bufs=1) as persist,
            tc.tile_pool(name="work", bufs=3) as work,
            tc.tile_pool(name="psum", bufs=1, space="PSUM") as psum,
            tc.tile_pool(name="psmall", bufs=2, space="PSUM") as psmall,
        ):
            # ---- persistent tiles (one DMA for all inputs) ----
            bigb = persist.tile([P, FINB], bf16)
            nc.gpsimd.dma_start(bigb[:], bigb_in[:])
            big = persist.tile([P, FIN], f32)
            nc.gpsimd.dma_start(big[:], big_in[:])
            embedB = bigb[:, 0 : NCHUNK * D]
            zb = bigb[:, NCHUNK * D :].rearrange("p (t f) -> p t f", t=T)
            # oh rows replicated on all partitions, layout (t, b, u)
            oh = big[0:B_LOC, 0 : T * B_LOC * U].rearrange("p (t b u) -> p t b u", t=T, b=B_LOC)
            eye4 = big[0:B_LOC, T * B_LOC * U :]  # (4, 4) identity
            eye_bc = eye4[:, :, None].broadcast_to([B_LOC, B_LOC, U])

            ones4 = persist.tile([B_LOC, P], bf16)
            nc.vector.memset(ones4[:], 1.0)
            consts = persist.tile([P, 3], f32)
            nc.vector.memset(consts[:, 0:1], 0.0)
            nc.vector.memset(consts[:, 1:2], 1.0)
            nc.vector.memset(consts[:, 2:3], EPS)
            czero, cone, ceps = consts[:, 0:1], consts[:, 1:2], consts[:, 2:3]

            attb = persist.tile([P, B_LOC, D], bf16)  # state, replicated on partitions
            nc.vector.memset(attb[:], 1.0 / D)
            assoc = persist.tile([P, B_LOC, NCHUNK, U], bf16)  # state
            nc.vector.memset(assoc[:], 0.0)

            probs_row = persist.tile([1, T, B_LOC, U], f32)

            # broadcast-view of embedB over batches: (P, B_LOC, NCHUNK, D)
            embed_bc = embedB.rearrange("p (c d) -> p c d", d=D)[:, None, :, :].broadcast_to([P, B_LOC, NCHUNK, D])

            for t in range(T):
                # -------- diff^rho chain on (P, B_LOC, NCHUNK, D) in one shot
                zrep = zb[:, t, :].rearrange("p (b d) -> p b d", d=D)[:, :, None, :].broadcast_to([P, B_LOC, NCHUNK, D])
                diff = work.tile([P, B_LOC, NCHUNK, D], bf16, tag="diff", bufs=2)
                nc.vector.tensor_tensor(diff[:], embed_bc, zrep, op=OP.subtract)
                nc.scalar.activation(diff[:], diff[:], AF.Abs, bias=czero)
                nc.scalar.activation(diff[:], diff[:], AF.Ln, bias=ceps)
                dpow = work.tile([P, B_LOC, NCHUNK, D], bf16, tag="dpow", bufs=2)
                nc.scalar.activation(dpow[:], diff[:], AF.Exp, bias=czero, scale=rho)

                # -------- q = sum_d att*dpow : TT mult + reduce
                qtmp = work.tile([P, B_LOC, NCHUNK, D], bf16, tag="qtmp", bufs=2)
                attb_bc = attb[:, :, None, :].broadcast_to([P, B_LOC, NCHUNK, D])
                nc.vector.tensor_tensor(qtmp[:], dpow[:], attb_bc, op=OP.mult)
                qall = work.tile([P, B_LOC, NCHUNK], f32, tag="qall")
                nc.vector.tensor_reduce(qall[:], qtmp[:], axis=mybir.AxisListType.X, op=OP.add)

                # -------- similarity acts on (P, B_LOC*NCHUNK)
                lnq = work.tile([P, B_LOC, NCHUNK], f32, tag="lnq")
                nc.scalar.activation(lnq[:], qall[:], AF.Ln, bias=ceps)
                dd = work.tile([P, B_LOC, NCHUNK], f32, tag="dd")
                nc.scalar.activation(dd[:], lnq[:], AF.Exp, bias=czero, scale=1.0 / rho)   # = d
                s_sim = work.tile([P, B_LOC, NCHUNK], bf16, tag="s_sim")
                nc.scalar.activation(s_sim[:], dd[:], AF.Exp, bias=czero, scale=-beta)     # = s (bf16)
                qp = work.tile([P, B_LOC, NCHUNK], bf16, tag="qp")
                nc.scalar.activation(qp[:], lnq[:], AF.Exp, bias=czero, scale=(1.0 - rho) / rho)


                # -------- x: M=4 packed matmuls (one per chunk), diag extract
                x_ps = psmall.tile([B_LOC, B_LOC, U], f32, tag="x_ps")
                for c in range(NCHUNK):
                    nc.tensor.matmul(x_ps[:, :, :],
                                     s_sim[:, :, c],
                                     assoc[:, :, c, :],
                                     start=(c == 0), stop=(c == NCHUNK - 1))
                # -------- teacher / dx on the cross tile (diag cols are the real x)
                pp = work.tile([B_LOC, B_LOC, U], f32, tag="pp")
                nc.scalar.activation(pp[:], x_ps[:], AF.Relu, bias=cone[:B_LOC, :])          # relu(x+1)
                mrow = work.tile([B_LOC, B_LOC, U], f32, tag="mrow")
                nc.scalar.activation(mrow[:], x_ps[:], AF.Relu, bias=cone[:B_LOC, :], scale=-1.0)  # relu(1-x)
                nc.vector.tensor_tensor(mrow[:], pp[:], mrow[:], op=OP.add)            # p+m
                nc.vector.tensor_tensor(mrow[:], mrow[:], oh[:, t, :, :], op=OP.mult)  # oh*(p+m)
                dxf = work.tile([B_LOC, B_LOC, U], f32, tag="dxf")
                nc.vector.tensor_tensor(dxf[:], pp[:], mrow[:], op=OP.subtract)
                # mask off-diagonal cross terms (bf16 out for single-pass matmuls)
                dxc = work.tile([B_LOC, B_LOC, U], bf16, tag="dxc")
                nc.vector.tensor_tensor(dxc[:], dxf[:], eye_bc, op=OP.mult)
                xm = work.tile([B_LOC, B_LOC, U], bf16, tag="xm")
                nc.vector.tensor_tensor(xm[:], x_ps[:], eye_bc, op=OP.mult)

                # -------- dx broadcast to (P, B_LOC, U): ones4^T @ dxm
                dxb_ps = psum.tile([P, B_LOC, U], f32, tag="dxb")
                nc.tensor.matmul(dxb_ps[:, :, :].rearrange("p b d -> p (b d)"),
                                 ones4[:], dxc[:].rearrange("p b u -> p (b u)"),
                                 start=True, stop=True)
                # x as a single row (1, B*U): ones4[:, :1]^T @ xm
                xrow_ps = psmall.tile([1, B_LOC, U], f32, tag="rowps")
                nc.tensor.matmul(xrow_ps[:, :, :].rearrange("p b u -> p (b u)"),
                                 ones4[:, 0:1], xm[:].rearrange("p b u -> p (b u)"),
                                 start=True, stop=True)

                # -------- softmax(temp*x) in row layout, per-b stable
                xr = work.tile([1, B_LOC, U], f32, tag="xr")
                nc.scalar.copy(xr[:], xrow_ps[:])
                mx4 = work.tile([1, B_LOC], f32, tag="mx4")
                nc.vector.tensor_reduce(mx4[:], xr[:], axis=mybir.AxisListType.X, op=OP.max)
                mx_bc = mx4[:, :, None].broadcast_to([1, B_LOC, U])
                xs_t = work.tile([1, B_LOC, U], f32, tag="xs_t")
                nc.vector.tensor_tensor(xs_t[:], xr[:], mx_bc, op=OP.subtract)
                er = work.tile([1, B_LOC, U], f32, tag="er")
                nc.scalar.activation(er[:], xs_t[:], AF.Exp, bias=czero[:1, :], scale=temperature)
                sm4 = work.tile([1, B_LOC], f32, tag="sm4")
                nc.vector.tensor_reduce(sm4[:], er[:], axis=mybir.AxisListType.X, op=OP.add)
                rc4 = work.tile([1, B_LOC], f32, tag="rc4")
                nc.vector.reciprocal(rc4[:], sm4[:])
                rc_bc = rc4[:, :, None].broadcast_to([1, B_LOC, U])
                nc.vector.tensor_tensor(probs_row[:, t, :, :], er[:], rc_bc, op=OP.mult)
                dxb = work.tile([P, B_LOC, U], bf16, tag="dxb_sb")
                nc.scalar.copy(dxb[:], dxb_ps[:])
                dxb_bc = dxb[:, :, None, :].broadcast_to([P, B_LOC, NCHUNK, U])

                # -------- y = sum_u assoc*dx : TT mult + reduce
                ytmp = work.tile([P, B_LOC, NCHUNK, U], bf16, tag="ytmp", bufs=2)
                nc.vector.tensor_tensor(ytmp[:], assoc[:], dxb_bc, op=OP.mult)
                yall = work.tile([P, B_LOC, NCHUNK], f32, tag="yall")
                nc.vector.tensor_reduce(yall[:], ytmp[:], axis=mybir.AxisListType.X, op=OP.add)

                # -------- c = -(beta/rho) * s * qp * y
                sqp = work.tile([P, B_LOC, NCHUNK], f32, tag="sqp")
                nc.vector.tensor_tensor(sqp[:], s_sim[:], qp[:], op=OP.mult)
                call_b16 = work.tile([P, B_LOC, NCHUNK], bf16, tag="call_b16")
                nc.vector.scalar_tensor_tensor(call_b16[:], yall[:], -beta / rho, sqp[:],
                                               op0=OP.mult, op1=OP.mult)

                # -------- g_att: M=4 packed matmuls + diag extract + att update
                gatt_ps = psmall.tile([B_LOC, B_LOC, D], f32, tag="gatt")
                for c in range(NCHUNK):
                    nc.tensor.matmul(gatt_ps[:, :, :],
                                     call_b16[:, :, c],
                                     dpow[:, :, c, :],
                                     start=(c == 0), stop=(c == NCHUNK - 1))
                gm = work.tile([B_LOC, B_LOC, D], bf16, tag="gm")
                nc.vector.tensor_tensor(gm[:], gatt_ps[:], eye_bc, op=OP.mult)
                grow_ps = psum.tile([P, B_LOC, D], f32, tag="grow")
                nc.tensor.matmul(grow_ps[:, :, :].rearrange("p b d -> p (b d)"),
                                 ones4[:], gm[:].rearrange("p b d -> p (b d)"),
                                 start=True, stop=True)
                nc.vector.scalar_tensor_tensor(attb[:], grow_ps[:], -lr_att, attb[:],
                                               op0=OP.mult, op1=OP.add)
                nc.vector.tensor_scalar_max(attb[:], attb[:], 0.0)

                # -------- assoc update: assoc += (-lr*s_bc) * dx_bu (2 big TTs)
                slr = work.tile([P, B_LOC, NCHUNK], bf16, tag="slr")
                nc.vector.tensor_scalar_mul(slr[:], s_sim[:], -lr_assoc)
                upd = work.tile([P, B_LOC, NCHUNK, U], bf16, tag="upd", bufs=2)
                slr_bc = slr[:, :, :, None].broadcast_to([P, B_LOC, NCHUNK, U])
                nc.vector.tensor_tensor(upd[:], slr_bc, dxb_bc, op=OP.mult)
                nc.vector.tensor_tensor(assoc[:], assoc[:], upd[:], op=OP.add)

            # -------- store: (1, T, B, U) row -> (B, T*U), one DMA per batch
            for b in range(B_LOC):
                nc.gpsimd.dma_start(out_ext[b : b + 1, :].rearrange("b (t u) -> b t u", t=T),
                                    probs_row[0:1, :, b, :])

    nc.compile()
    return nc


def _prep_in_maps(stimulus_set, label_idx, embed):
    embedB = embed.reshape(NCHUNK, P, D).transpose(1, 0, 2).reshape(P, NCHUNK * D)
    z = embed[stimulus_set]  # (B, T, D)
    onehot = np.zeros((B, T, U), dtype=np.float32)
    bi, ti = np.meshgrid(np.arange(B), np.arange(T), indexing="ij")
    onehot[bi, ti, label_idx] = 1.0
    in_maps = []
    for i in range(N_CORES):
        bs = slice(i * B_LOC, (i + 1) * B_LOC)
        zc = z[bs].transpose(1, 0, 2).reshape(1, T * B_LOC * D)
        zbcast = np.broadcast_to(zc, (P, T * B_LOC * D))
        ohrow = onehot[bs].transpose(1, 0, 2).reshape(1, T * B_LOC * U)
        ohfull = np.broadcast_to(ohrow, (P, T * B_LOC * U))
        eyefull = np.zeros((P, B_LOC), dtype=np.float32)
        eyefull[:B_LOC, :] = np.eye(B_LOC, dtype=np.float32)
        import ml_dtypes
        bigb = np.concatenate([embedB, zbcast], axis=1).astype(ml_dtypes.bfloat16)
        big = np.concatenate([ohfull, eyefull], axis=1).astype(np.float32)
        in_maps.append({"bigb": np.ascontiguousarray(bigb),
                        "big": np.ascontiguousarray(big)})
    return in_maps


def kernel(stimulus_set, label_idx, embed, rho, temperature, lr_attention, lr_association, beta):
    from concourse.bass_utils import run_bass_kernel_spmd

    stimulus_set = np.asarray(stimulus_set)
    label_idx = np.asarray(label_idx)
    embed = np.asarray(embed, dtype=np.float32)
    key = (float(rho), float(temperature), float(lr_attention),
           float(lr_association), float(beta))
    if key not in _cache:
        _cache[key] = _build(*key)
    nc = _cache[key]
    in_maps = _prep_in_maps(stimulus_set, label_idx, embed)
    res = run_bass_kernel_spmd(nc, in_maps, core_ids=list(range(N_CORES)))
    outs = [res.results[i]["out"].reshape(B_LOC, T, U) for i in range(N_CORES)]
    return np.concatenate(outs, axis=0)


def _install_ntff_hook():
    import sys, types, ctypes, contextlib
    if "antenv.axon_hooks" in sys.modules:
        return
    import antenv
    mod = types.ModuleType("antenv.axon_hooks")
    mod._hook = None
    def set_axon_ntff_profile_hook(h):
        mod._hook = h
    def get_axon_ntff_profile_hook():
        return mod._hook
    mod.set_axon_ntff_profile_hook = set_axon_ntff_profile_hook
    mod.get_axon_ntff_profile_hook = get_axon_ntff_profile_hook
    sys.modules["antenv.axon_hooks"] = mod
    antenv.axon_hooks = mod

    lib = ctypes.CDLL("/opt/axon/libaxon_pjrt.so")
    if not hasattr(lib, "axon_start_nrt_profile"):
        return
    lib.axon_start_nrt_profile.argtypes = [ctypes.POINTER(ctypes.c_int64), ctypes.c_size_t]
    lib.axon_start_nrt_profile.restype = ctypes.c_int64
    lib.axon_stop_nrt_profile.argtypes = [ctypes.c_char_p]
    lib.axon_stop_nrt_profile.restype = ctypes.c_int64

    @contextlib.contextmanager
    def _hook(output_dir, device_ids):
        import jax
        jax.devices()
        if device_ids:
            ids = (ctypes.c_int64 * len(device_ids))(*device_ids)
            rc = lib.axon_start_nrt_profile(ids, len(device_ids))
        else:
            rc = lib.axon_start_nrt_profile(None, 0)
        if rc != 0:
            raise RuntimeError(f"axon_start_nrt_profile rc={rc}")
        try:
            yield
        finally:
            n = lib.axon_stop_nrt_profile(str(output_dir).encode())
            print(f"profile: {n} file(s) written to {output_dir}")

    set_axon_ntff_profile_hook(_hook)


def kernel_traced(**inputs):
    """Like kernel() but runs with NTFF tracing; returns (out, exec_time_ns, tmpdir)."""
    import tempfile
    _install_ntff_hook()
    from concourse.bass_utils import run_bass_kernel_spmd

    key = (float(inputs["rho"]), float(inputs["temperature"]), float(inputs["lr_attention"]),
           float(inputs["lr_association"]), float(inputs["beta"]))
    if key not in _cache:
        _cache[key] = _build(*key)
    nc = _cache[key]
    in_maps = _prep_in_maps(np.asarray(inputs["stimulus_set"]), np.asarray(inputs["label_idx"]),
                            np.asarray(inputs["embed"], dtype=np.float32))
    tmpdir = tempfile.mkdtemp(prefix="alcove_trace_")
    res = run_bass_kernel_spmd(nc, in_maps, core_ids=list(range(N_CORES)), trace=True, tmpdir=tmpdir)
    outs = [res.results[i]["out"].reshape(B_LOC, T, U) for i in range(N_CORES)]
    return np.concatenate(outs, axis=0), res.exec_time_ns, tmpdir


# revision 31
# speedup vs baseline: 1.2404x; 1.1300x over previous
"""ALCOVE cell Bass kernel for 8 TRN2 NeuronCores (data-parallel over batch).

B=32, T=16, N_RBF=1024, N_DIM=64, UNITS=64. 4 batches per core.

Layout: R=1024 on partitions as 8 chunks of 128; per-batch row data
(att, x, dx, g_att) on partition 0 as (1, B_LOC*64) rows (PE base-partition
rule); partition broadcasts via K=1 ones-matmul. Big elementwise work runs
on (128, B_LOC*NCHUNK*64) = (128, 2048) tiles in single instructions;
contractions over the free dim use TT-mult + tensor_reduce; contractions
over partitions use M=1 accumulating matmuls.
"""

import numpy as np

B, T, R, D, U = 32, 16, 1024, 64, 64
NCHUNK, P = 8, 128
EPS = 1e-6
N_CORES = 8
B_LOC = B // N_CORES  # 4

_cache = {}


def _patch_act_tables():
    """Make every activation resolve to natural_log_exp_and_others (it
    contains abs/ln/exp/relu/copy/identity/square) so the kernel needs a
    single ACT table load instead of thrashing between sets."""
    import concourse.bacc as bacc_mod
    from concourse.hw_specs import get_activation_tables as _gat

    if getattr(bacc_mod.get_activation_tables, "_alcove_patched", False):
        return

    def patched(arch):
        t = _gat(arch)
        keep = t["natural_log_exp_and_others"]
        out = {}
        for name, fns in t.items():
            out[name] = fns if name == "natural_log_exp_and_others" else (fns - keep)
        return out

    patched._alcove_patched = True
    bacc_mod.get_activation_tables = patched


def _build(rho, temperature, lr_att, lr_assoc, beta):
    import concourse.bass as bass
    import concourse.tile as tile
    from concourse import bacc, mybir

    _patch_act_tables()

    f32 = mybir.dt.float32
    bf16 = mybir.dt.bfloat16
    AF = mybir.ActivationFunctionType
    OP = mybir.AluOpType

    nc = bacc.Bacc("TRN2", target_bir_lowering=False, debug=False, num_devices=N_CORES)
    # packed bf16 input: [embedB (512) | zbcast (4096)]; f32 input: [oh (1024) | eye4 (4)]
    FINB = NCHUNK * D + T * B_LOC * D
    FIN = T * B_LOC * U + B_LOC
    bigb_in = nc.declare_dram_parameter("bigb", [P, FINB], bf16, isOutput=False)
    big_in = nc.declare_dram_parameter("big", [P, FIN], f32, isOutput=False)
    out_ext = nc.declare_dram_parameter("out", [B_LOC, T * U], f32, isOutput=True)

    with tile.TileContext(nc) as tc:
        with (
            tc.tile_pool(name="persist", bufs=1) as persist,
            tc.tile_pool(name="work", bufs=3) as work,
            tc.tile_pool(name="psum", bufs=1, space="PSUM") as psum,
            tc.tile_pool(name="psmall", bufs=2, space="PSUM") as psmall,
        ):
            # ---- persistent tiles (one DMA for all inputs) ----
            bigb = persist.tile([P, FINB], bf16)
            nc.gpsimd.dma_start(bigb[:], bigb_in[:])
            big = persist.tile([P, FIN], f32)
            nc.gpsimd.dma_start(big[:], big_in[:])
            embedB = bigb[:, 0 : NCHUNK * D]
            zb = bigb[:, NCHUNK * D :].rearrange("p (t f) -> p t f", t=T)
            # oh rows replicated on all partitions, layout (t, b, u)
            oh = big[0:B_LOC, 0 : T * B_LOC * U].rearrange("p (t b u) -> p t b u", t=T, b=B_LOC)
            eye4 = big[0:B_LOC, T * B_LOC * U :]  # (4, 4) identity
            eye_bc = eye4[:, :, None].broadcast_to([B_LOC, B_LOC, U])

            ones4 = persist.tile([B_LOC, P], bf16)
            nc.vector.memset(ones4[:], 1.0)
            consts = persist.tile([P, 3], f32)
            nc.vector.memset(consts[:, 0:1], 0.0)
            nc.vector.memset(consts[:, 1:2], 1.0)
            nc.vector.memset(consts[:, 2:3], EPS)
            czero, cone, ceps = consts[:, 0:1], consts[:, 1:2], consts[:, 2:3]

            attb = persist.tile([P, B_LOC, D], bf16)  # state, replicated on partitions
            nc.vector.memset(attb[:], 1.0 / D)
            assoc = persist.tile([P, B_LOC, NCHUNK, U], bf16)  # state
            nc.vector.memset(assoc[:], 0.0)

            probs_row = persist.tile([1, T, B_LOC, U], f32)

            # broadcast-view of embedB over batches: (P, B_LOC, NCHUNK, D)
            embed_bc = embedB.rearrange("p (c d) -> p c d", d=D)[:, None, :, :].broadcast_to([P, B_LOC, NCHUNK, D])

            for t in range(T):
                # -------- diff^rho chain on (P, B_LOC, NCHUNK, D) in one shot
                zrep = zb[:, t, :].rearrange("p (b d) -> p b d", d=D)[:, :, None, :].broadcast_to([P, B_LOC, NCHUNK, D])
                diff = work.tile([P, B_LOC, NCHUNK, D], bf16, tag="diff", bufs=2)
                nc.vector.tensor_tensor(diff[:], embed_bc, zrep, op=OP.subtract)
                nc.scalar.activation(diff[:], diff[:], AF.Abs, bias=czero)
                nc.scalar.activation(diff[:], diff[:], AF.Ln, bias=ceps)
                dpow = work.tile([P, B_LOC, NCHUNK, D], bf16, tag="dpow", bufs=2)
                nc.scalar.activation(dpow[:], diff[:], AF.Exp, bias=czero, scale=rho)

                # -------- q = sum_d att*dpow : TT mult + reduce
                qtmp = work.tile([P, B_LOC, NCHUNK, D], bf16, tag="qtmp", bufs=2)
                attb_bc = attb[:, :, None, :].broadcast_to([P, B_LOC, NCHUNK, D])
                nc.vector.tensor_tensor(qtmp[:], dpow[:], attb_bc, op=OP.mult)
                qall = work.tile([P, B_LOC, NCHUNK], f32, tag="qall")
                nc.vector.tensor_reduce(qall[:], qtmp[:], axis=mybir.AxisListType.X, op=OP.add)

                # -------- similarity acts on (P, B_LOC*NCHUNK)
                lnq = work.tile([P, B_LOC, NCHUNK], f32, tag="lnq")
                nc.scalar.activation(lnq[:], qall[:], AF.Ln, bias=ceps)
                s_sim = work.tile([P, B_LOC, NCHUNK], f32, tag="s_sim")
                nc.scalar.activation(s_sim[:], lnq[:], AF.Exp, bias=czero, scale=1.0 / rho)  # = d
                nc.scalar.activation(s_sim[:], s_sim[:], AF.Exp, bias=czero, scale=-beta)    # = s
                qp = work.tile([P, B_LOC, NCHUNK], f32, tag="qp")
                nc.scalar.activation(qp[:], lnq[:], AF.Exp, bias=czero, scale=(1.0 - rho) / rho)
                s_b16 = work.tile([P, B_LOC, NCHUNK], bf16, tag="s_b16")
                nc.vector.tensor_copy(s_b16[:], s_sim[:])

                # -------- x: M=4 packed matmuls (one per chunk), diag extract
                x_ps = psmall.tile([B_LOC, B_LOC, U], f32, tag="x_ps")
                for c in range(NCHUNK):
                    nc.tensor.matmul(x_ps[:, :, :],
                                     s_b16[:, :, c],
                                     assoc[:, :, c, :],
                                     start=(c == 0), stop=(c == NCHUNK - 1))
                # -------- teacher / dx on the cross tile (diag cols are the real x)
                pp = work.tile([B_LOC, B_LOC, U], f32, tag="pp")
                nc.scalar.activation(pp[:], x_ps[:], AF.Relu, bias=cone[:B_LOC, :])          # relu(x+1)
                mrow = work.tile([B_LOC, B_LOC, U], f32, tag="mrow")
                nc.scalar.activation(mrow[:], x_ps[:], AF.Relu, bias=cone[:B_LOC, :], scale=-1.0)  # relu(1-x)
                nc.vector.tensor_tensor(mrow[:], pp[:], mrow[:], op=OP.add)            # p+m
                nc.vector.tensor_tensor(mrow[:], mrow[:], oh[:, t, :, :], op=OP.mult)  # oh*(p+m)
                dxf = work.tile([B_LOC, B_LOC, U], f32, tag="dxf")
                nc.vector.tensor_tensor(dxf[:], pp[:], mrow[:], op=OP.subtract)
                # mask off-diagonal cross terms (bf16 out for single-pass matmuls)
                dxc = work.tile([B_LOC, B_LOC, U], bf16, tag="dxc")
                nc.vector.tensor_tensor(dxc[:], dxf[:], eye_bc, op=OP.mult)
                xm = work.tile([B_LOC, B_LOC, U], bf16, tag="xm")
                nc.vector.tensor_tensor(xm[:], x_ps[:], eye_bc, op=OP.mult)

                # -------- dx broadcast to (P, B_LOC, U): ones4^T @ dxm
                dxb_ps = psum.tile([P, B_LOC, U], f32, tag="dxb")
                nc.tensor.matmul(dxb_ps[:, :, :].rearrange("p b d -> p (b d)"),
                                 ones4[:], dxc[:].rearrange("p b u -> p (b u)"),
                                 start=True, stop=True)
                # x as a single row (1, B*U): ones4[:, :1]^T @ xm
                xrow_ps = psmall.tile([1, B_LOC, U], f32, tag="rowps")
                nc.tensor.matmul(xrow_ps[:, :, :].rearrange("p b u -> p (b u)"),
                                 ones4[:, 0:1], xm[:].rearrange("p b u -> p (b u)"),
                                 start=True, stop=True)

                # -------- softmax(temp*x) in row layout, per-b stable
                xr = work.tile([1, B_LOC, U], f32, tag="xr")
                nc.scalar.copy(xr[:], xrow_ps[:])
                mx4 = work.tile([1, B_LOC], f32, tag="mx4")
                nc.vector.tensor_reduce(mx4[:], xr[:], axis=mybir.AxisListType.X, op=OP.max)
                mx_bc = mx4[:, :, None].broadcast_to([1, B_LOC, U])
                xs_t = work.tile([1, B_LOC, U], f32, tag="xs_t")
                nc.vector.tensor_tensor(xs_t[:], xr[:], mx_bc, op=OP.subtract)
                er = work.tile([1, B_LOC, U], f32, tag="er")
                nc.scalar.activation(er[:], xs_t[:], AF.Exp, bias=czero[:1, :], scale=temperature)
                sm4 = work.tile([1, B_LOC], f32, tag="sm4")
                nc.vector.tensor_reduce(sm4[:], er[:], axis=mybir.AxisListType.X, op=OP.add)
                rc4 = work.tile([1, B_LOC], f32, tag="rc4")
                nc.vector.reciprocal(rc4[:], sm4[:])
                rc_bc = rc4[:, :, None].broadcast_to([1, B_LOC, U])
                nc.vector.tensor_tensor(probs_row[:, t, :, :], er[:], rc_bc, op=OP.mult)
                dxb = work.tile([P, B_LOC, U], bf16, tag="dxb_sb")
                nc.scalar.copy(dxb[:], dxb_ps[:])
                dxb_bc = dxb[:, :, None, :].broadcast_to([P, B_LOC, NCHUNK, U])

                # -------- y = sum_u assoc*dx : TT mult + reduce
                ytmp = work.tile([P, B_LOC, NCHUNK, U], bf16, tag="ytmp", bufs=2)
                nc.vector.tensor_tensor(ytmp[:], assoc[:], dxb_bc, op=OP.mult)
                yall = work.tile([P, B_LOC, NCHUNK], f32, tag="yall")
                nc.vector.tensor_reduce(yall[:], ytmp[:], axis=mybir.AxisListType.X, op=OP.add)

                # -------- c = -(beta/rho) * s * qp * y
                call = work.tile([P, B_LOC, NCHUNK], f32, tag="call")
                nc.vector.tensor_tensor(call[:], s_sim[:], qp[:], op=OP.mult)
                nc.vector.scalar_tensor_tensor(call[:], yall[:], -beta / rho, call[:],
                                               op0=OP.mult, op1=OP.mult)
                call_b16 = work.tile([P, B_LOC, NCHUNK], bf16, tag="call_b16")
                nc.vector.tensor_copy(call_b16[:], call[:])

                # -------- g_att: M=4 packed matmuls + diag extract + att update
                gatt_ps = psmall.tile([B_LOC, B_LOC, D], f32, tag="gatt")
                for c in range(NCHUNK):
                    nc.tensor.matmul(gatt_ps[:, :, :],
                                     call_b16[:, :, c],
                                     dpow[:, :, c, :],
                                     start=(c == 0), stop=(c == NCHUNK - 1))
                gm = work.tile([B_LOC, B_LOC, D], bf16, tag="gm")
                nc.vector.tensor_tensor(gm[:], gatt_ps[:], eye_bc, op=OP.mult)
                grow_ps = psum.tile([P, B_LOC, D], f32, tag="grow")
                nc.tensor.matmul(grow_ps[:, :, :].rearrange("p b d -> p (b d)"),
                                 ones4[:], gm[:].rearrange("p b d -> p (b d)"),
                                 start=True, stop=True)
                nc.vector.scalar_tensor_tensor(attb[:], grow_ps[:], -lr_att, attb[:],
                                               op0=OP.mult, op1=OP.add)
                nc.vector.tensor_scalar_max(attb[:], attb[:], 0.0)

                # -------- assoc update: assoc += (-lr*s_bc) * dx_bu (2 big TTs)
                slr = work.tile([P, B_LOC, NCHUNK], bf16, tag="slr")
                nc.vector.tensor_scalar_mul(slr[:], s_b16[:], -lr_assoc)
                upd = work.tile([P, B_LOC, NCHUNK, U], bf16, tag="upd", bufs=2)
                slr_bc = slr[:, :, :, None].broadcast_to([P, B_LOC, NCHUNK, U])
                nc.vector.tensor_tensor(upd[:], slr_bc, dxb_bc, op=OP.mult)
                nc.vector.tensor_tensor(assoc[:], assoc[:], upd[:], op=OP.add)

            # -------- store: (1, T, B, U) row -> (B, T*U), one DMA per batch
            for b in range(B_LOC):
                nc.gpsimd.dma_start(out_ext[b : b + 1, :].rearrange("b (t u) -> b t u", t=T),
                                    probs_row[0:1, :, b, :])

    nc.compile()
    return nc


def _prep_in_maps(stimulus_set, label_idx, embed):
    embedB = embed.reshape(NCHUNK, P, D).transpose(1, 0, 2).reshape(P, NCHUNK * D)
    z = embed[stimulus_set]  # (B, T, D)
    onehot = np.zeros((B, T, U), dtype=np.float32)
    bi, ti = np.meshgrid(np.arange(B), np.arange(T), indexing="ij")
    onehot[bi, ti, label_idx] = 1.0
    in_maps = []
    for i in range(N_CORES):
        bs = slice(i * B_LOC, (i + 1) * B_LOC)
        zc = z[bs].transpose(1, 0, 2).reshape(1, T * B_LOC * D)
        zbcast = np.broadcast_to(zc, (P, T * B_LOC * D))
        ohrow = onehot[bs].transpose(1, 0, 2).reshape(1, T * B_LOC * U)
        ohfull = np.broadcast_to(ohrow, (P, T * B_LOC * U))
        eyefull = np.zeros((P, B_LOC), dtype=np.float32)
        eyefull[:B_LOC, :] = np.eye(B_LOC, dtype=np.float32)
        import ml_dtypes
        bigb = np.concatenate([embedB, zbcast], axis=1).astype(ml_dtypes.bfloat16)
        big = np.concatenate([ohfull, eyefull], axis=1).astype(np.float32)
        in_maps.append({"bigb": np.ascontiguousarray(bigb),
                        "big": np.ascontiguousarray(big)})
    return in_maps


def kernel(stimulus_set, label_idx, embed, rho, temperature, lr_attention, lr_association, beta):
    from concourse.bass_utils import run_bass_kernel_spmd

    stimulus_set = np.asarray(stimulus_set)
    label_idx = np.asarray(label_idx)
    embed = np.asarray(embed, dtype=np.float32)
    key = (float(rho), float(temperature), float(lr_attention),
           float(lr_association), float(beta))
    if key not in _cache:
        _cache[key] = _build(*key)
    nc = _cache[key]
    in_maps = _prep_in_maps(stimulus_set, label_idx, embed)
    res = run_bass_kernel_spmd(nc, in_maps, core_ids=list(range(N_CORES)))
    outs = [res.results[i]["out"].reshape(B_LOC, T, U) for i in range(N_CORES)]
    return np.concatenate(outs, axis=0)


def _install_ntff_hook():
    import sys, types, ctypes, contextlib
    if "antenv.axon_hooks" in sys.modules:
        return
    import antenv
    mod = types.ModuleType("antenv.axon_hooks")
    mod._hook = None
    def set_axon_ntff_profile_hook(h):
        mod._hook = h
    def get_axon_ntff_profile_hook():
        return mod._hook
    mod.set_axon_ntff_profile_hook = set_axon_ntff_profile_hook
    mod.get_axon_ntff_profile_hook = get_axon_ntff_profile_hook
    sys.modules["antenv.axon_hooks"] = mod
    antenv.axon_hooks = mod

    lib = ctypes.CDLL("/opt/axon/libaxon_pjrt.so")
    if not hasattr(lib, "axon_start_nrt_profile"):
        return
    lib.axon_start_nrt_profile.argtypes = [ctypes.POINTER(ctypes.c_int64), ctypes.c_size_t]
    lib.axon_start_nrt_profile.restype = ctypes.c_int64
    lib.axon_stop_nrt_profile.argtypes = [ctypes.c_char_p]
    lib.axon_stop_nrt_profile.restype = ctypes.c_int64

    @contextlib.contextmanager
    def _hook(output_dir, device_ids):
        import jax
        jax.devices()
        if device_ids:
            ids = (ctypes.c_int64 * len(device_ids))(*device_ids)
            rc = lib.axon_start_nrt_profile(ids, len(device_ids))
        else:
            rc = lib.axon_start_nrt_profile(None, 0)
        if rc != 0:
            raise RuntimeError(f"axon_start_nrt_profile rc={rc}")
        try:
            yield
        finally:
            n = lib.axon_stop_nrt_profile(str(output_dir).encode())
            print(f"profile: {n} file(s) written to {output_dir}")

    set_axon_ntff_profile_hook(_hook)


def kernel_traced(**inputs):
    """Like kernel() but runs with NTFF tracing; returns (out, exec_time_ns, tmpdir)."""
    import tempfile
    _install_ntff_hook()
    from concourse.bass_utils import run_bass_kernel_spmd

    key = (float(inputs["rho"]), float(inputs["temperature"]), float(inputs["lr_attention"]),
           float(inputs["lr_association"]), float(inputs["beta"]))
    if key not in _cache:
        _cache[key] = _build(*key)
    nc = _cache[key]
    in_maps = _prep_in_maps(np.asarray(inputs["stimulus_set"]), np.asarray(inputs["label_idx"]),
                            np.asarray(inputs["embed"], dtype=np.float32))
    tmpdir = tempfile.mkdtemp(prefix="alcove_trace_")
    res = run_bass_kernel_spmd(nc, in_maps, core_ids=list(range(N_CORES)), trace=True, tmpdir=tmpdir)
    outs = [res.results[i]["out"].reshape(B_LOC, T, U) for i in range(N_CORES)]
    return np.concatenate(outs, axis=0), res.exec_time_ns, tmpdir


# revision 32
# speedup vs baseline: 1.2718x; 1.0253x over previous
"""ALCOVE cell Bass kernel for 8 TRN2 NeuronCores (data-parallel over batch).

B=32, T=16, N_RBF=1024, N_DIM=64, UNITS=64. 4 batches per core.

Layout: R=1024 on partitions as 8 chunks of 128; per-batch row data
(att, x, dx, g_att) on partition 0 as (1, B_LOC*64) rows (PE base-partition
rule); partition broadcasts via K=1 ones-matmul. Big elementwise work runs
on (128, B_LOC*NCHUNK*64) = (128, 2048) tiles in single instructions;
contractions over the free dim use TT-mult + tensor_reduce; contractions
over partitions use M=1 accumulating matmuls.
"""

import numpy as np

B, T, R, D, U = 32, 16, 1024, 64, 64
NCHUNK, P = 8, 128
EPS = 1e-6
N_CORES = 8
B_LOC = B // N_CORES  # 4

_cache = {}


def _patch_act_tables():
    """Make every activation resolve to natural_log_exp_and_others (it
    contains abs/ln/exp/relu/copy/identity/square) so the kernel needs a
    single ACT table load instead of thrashing between sets."""
    import concourse.bacc as bacc_mod
    from concourse.hw_specs import get_activation_tables as _gat

    if getattr(bacc_mod.get_activation_tables, "_alcove_patched", False):
        return

    def patched(arch):
        t = _gat(arch)
        keep = t["natural_log_exp_and_others"]
        out = {}
        for name, fns in t.items():
            out[name] = fns if name == "natural_log_exp_and_others" else (fns - keep)
        return out

    patched._alcove_patched = True
    bacc_mod.get_activation_tables = patched


def _build(rho, temperature, lr_att, lr_assoc, beta):
    import concourse.bass as bass
    import concourse.tile as tile
    from concourse import bacc, mybir

    _patch_act_tables()

    f32 = mybir.dt.float32
    bf16 = mybir.dt.bfloat16
    AF = mybir.ActivationFunctionType
    OP = mybir.AluOpType

    nc = bacc.Bacc("TRN2", target_bir_lowering=False, debug=False, num_devices=N_CORES)
    # packed bf16 input: [embedB (512) | zbcast (4096)]; f32 input: [oh (1024) | eye4 (4)]
    FINB = NCHUNK * D + T * B_LOC * D
    FIN = T * B_LOC * U + B_LOC
    bigb_in = nc.declare_dram_parameter("bigb", [P, FINB], bf16, isOutput=False)
    big_in = nc.declare_dram_parameter("big", [P, FIN], f32, isOutput=False)
    out_ext = nc.declare_dram_parameter("out", [B_LOC, T * U], f32, isOutput=True)

    with tile.TileContext(nc) as tc:
        with (
            tc.tile_pool(name="persist", bufs=1) as persist,
            tc.tile_pool(name="work", bufs=3) as work,
            tc.tile_pool(name="psum", bufs=1, space="PSUM") as psum,
            tc.tile_pool(name="psmall", bufs=2, space="PSUM") as psmall,
        ):
            # ---- persistent tiles (one DMA for all inputs) ----
            bigb = persist.tile([P, FINB], bf16)
            nc.gpsimd.dma_start(bigb[:], bigb_in[:])
            big = persist.tile([P, FIN], f32)
            nc.gpsimd.dma_start(big[:], big_in[:])
            embedB = bigb[:, 0 : NCHUNK * D]
            zb = bigb[:, NCHUNK * D :].rearrange("p (t f) -> p t f", t=T)
            # oh rows replicated on all partitions, layout (t, b, u)
            oh = big[0:B_LOC, 0 : T * B_LOC * U].rearrange("p (t b u) -> p t b u", t=T, b=B_LOC)
            eye4 = big[0:B_LOC, T * B_LOC * U :]  # (4, 4) identity
            eye_bc = eye4[:, :, None].broadcast_to([B_LOC, B_LOC, U])

            ones4 = persist.tile([B_LOC, P], bf16)
            nc.vector.memset(ones4[:], 1.0)
            consts = persist.tile([P, 3], f32)
            nc.vector.memset(consts[:, 0:1], 0.0)
            nc.vector.memset(consts[:, 1:2], 1.0)
            nc.vector.memset(consts[:, 2:3], EPS)
            czero, cone, ceps = consts[:, 0:1], consts[:, 1:2], consts[:, 2:3]

            attb = persist.tile([P, B_LOC, D], bf16)  # state, replicated on partitions
            nc.vector.memset(attb[:], 1.0 / D)
            assoc = persist.tile([P, B_LOC, NCHUNK, U], bf16)  # state
            nc.vector.memset(assoc[:], 0.0)

            probs_row = persist.tile([1, T, B_LOC, U], f32)

            # broadcast-view of embedB over batches: (P, B_LOC, NCHUNK, D)
            embed_bc = embedB.rearrange("p (c d) -> p c d", d=D)[:, None, :, :].broadcast_to([P, B_LOC, NCHUNK, D])

            for t in range(T):
                # -------- diff^rho chain on (P, B_LOC, NCHUNK, D) in one shot
                zrep = zb[:, t, :].rearrange("p (b d) -> p b d", d=D)[:, :, None, :].broadcast_to([P, B_LOC, NCHUNK, D])
                diff = work.tile([P, B_LOC, NCHUNK, D], bf16, tag="diff", bufs=4)
                nc.vector.tensor_tensor(diff[:], embed_bc, zrep, op=OP.subtract)
                nc.scalar.activation(diff[:], diff[:], AF.Abs, bias=czero)
                nc.scalar.activation(diff[:], diff[:], AF.Ln, bias=ceps)
                dpow = work.tile([P, B_LOC, NCHUNK, D], bf16, tag="dpow", bufs=4)
                nc.scalar.activation(dpow[:], diff[:], AF.Exp, bias=czero, scale=rho)

                # -------- q = sum_d att*dpow : TT mult + reduce
                qtmp = work.tile([P, B_LOC, NCHUNK, D], bf16, tag="qtmp", bufs=3)
                attb_bc = attb[:, :, None, :].broadcast_to([P, B_LOC, NCHUNK, D])
                nc.vector.tensor_tensor(qtmp[:], dpow[:], attb_bc, op=OP.mult)
                qall = work.tile([P, B_LOC, NCHUNK], f32, tag="qall")
                nc.vector.tensor_reduce(qall[:], qtmp[:], axis=mybir.AxisListType.X, op=OP.add)

                # -------- similarity acts on (P, B_LOC*NCHUNK)
                lnq = work.tile([P, B_LOC, NCHUNK], f32, tag="lnq")
                nc.scalar.activation(lnq[:], qall[:], AF.Ln, bias=ceps)
                s_sim = work.tile([P, B_LOC, NCHUNK], f32, tag="s_sim")
                nc.scalar.activation(s_sim[:], lnq[:], AF.Exp, bias=czero, scale=1.0 / rho)  # = d
                nc.scalar.activation(s_sim[:], s_sim[:], AF.Exp, bias=czero, scale=-beta)    # = s
                qp = work.tile([P, B_LOC, NCHUNK], f32, tag="qp")
                nc.scalar.activation(qp[:], lnq[:], AF.Exp, bias=czero, scale=(1.0 - rho) / rho)
                s_b16 = work.tile([P, B_LOC, NCHUNK], bf16, tag="s_b16")
                nc.vector.tensor_copy(s_b16[:], s_sim[:])

                # -------- x: M=4 packed matmuls (one per chunk), diag extract
                x_ps = psmall.tile([B_LOC, B_LOC, U], f32, tag="x_ps")
                for c in range(NCHUNK):
                    nc.tensor.matmul(x_ps[:, :, :],
                                     s_b16[:, :, c],
                                     assoc[:, :, c, :],
                                     start=(c == 0), stop=(c == NCHUNK - 1))
                # -------- teacher / dx on the cross tile (diag cols are the real x)
                pp = work.tile([B_LOC, B_LOC, U], f32, tag="pp")
                nc.scalar.activation(pp[:], x_ps[:], AF.Relu, bias=cone[:B_LOC, :])          # relu(x+1)
                mrow = work.tile([B_LOC, B_LOC, U], f32, tag="mrow")
                nc.scalar.activation(mrow[:], x_ps[:], AF.Relu, bias=cone[:B_LOC, :], scale=-1.0)  # relu(1-x)
                nc.vector.tensor_tensor(mrow[:], pp[:], mrow[:], op=OP.add)            # p+m
                nc.vector.tensor_tensor(mrow[:], mrow[:], oh[:, t, :, :], op=OP.mult)  # oh*(p+m)
                dxf = work.tile([B_LOC, B_LOC, U], f32, tag="dxf")
                nc.vector.tensor_tensor(dxf[:], pp[:], mrow[:], op=OP.subtract)
                # mask off-diagonal cross terms (bf16 out for single-pass matmuls)
                dxc = work.tile([B_LOC, B_LOC, U], bf16, tag="dxc")
                nc.vector.tensor_tensor(dxc[:], dxf[:], eye_bc, op=OP.mult)
                xm = work.tile([B_LOC, B_LOC, U], bf16, tag="xm")
                nc.vector.tensor_tensor(xm[:], x_ps[:], eye_bc, op=OP.mult)

                # -------- dx broadcast to (P, B_LOC, U): ones4^T @ dxm
                dxb_ps = psum.tile([P, B_LOC, U], f32, tag="dxb")
                nc.tensor.matmul(dxb_ps[:, :, :].rearrange("p b d -> p (b d)"),
                                 ones4[:], dxc[:].rearrange("p b u -> p (b u)"),
                                 start=True, stop=True)
                # x as a single row (1, B*U): ones4[:, :1]^T @ xm
                xrow_ps = psmall.tile([1, B_LOC, U], f32, tag="rowps")
                nc.tensor.matmul(xrow_ps[:, :, :].rearrange("p b u -> p (b u)"),
                                 ones4[:, 0:1], xm[:].rearrange("p b u -> p (b u)"),
                                 start=True, stop=True)

                # -------- softmax(temp*x) in row layout, per-b stable
                xr = work.tile([1, B_LOC, U], f32, tag="xr")
                nc.scalar.copy(xr[:], xrow_ps[:])
                mx4 = work.tile([1, B_LOC], f32, tag="mx4")
                nc.vector.tensor_reduce(mx4[:], xr[:], axis=mybir.AxisListType.X, op=OP.max)
                mx_bc = mx4[:, :, None].broadcast_to([1, B_LOC, U])
                xs_t = work.tile([1, B_LOC, U], f32, tag="xs_t")
                nc.vector.tensor_tensor(xs_t[:], xr[:], mx_bc, op=OP.subtract)
                er = work.tile([1, B_LOC, U], f32, tag="er")
                nc.scalar.activation(er[:], xs_t[:], AF.Exp, bias=czero[:1, :], scale=temperature)
                sm4 = work.tile([1, B_LOC], f32, tag="sm4")
                nc.vector.tensor_reduce(sm4[:], er[:], axis=mybir.AxisListType.X, op=OP.add)
                rc4 = work.tile([1, B_LOC], f32, tag="rc4")
                nc.vector.reciprocal(rc4[:], sm4[:])
                rc_bc = rc4[:, :, None].broadcast_to([1, B_LOC, U])
                nc.vector.tensor_tensor(probs_row[:, t, :, :], er[:], rc_bc, op=OP.mult)
                dxb = work.tile([P, B_LOC, U], bf16, tag="dxb_sb")
                nc.scalar.copy(dxb[:], dxb_ps[:])
                dxb_bc = dxb[:, :, None, :].broadcast_to([P, B_LOC, NCHUNK, U])

                # -------- y = sum_u assoc*dx : TT mult + reduce
                ytmp = work.tile([P, B_LOC, NCHUNK, U], bf16, tag="ytmp", bufs=3)
                nc.vector.tensor_tensor(ytmp[:], assoc[:], dxb_bc, op=OP.mult)
                yall = work.tile([P, B_LOC, NCHUNK], f32, tag="yall")
                nc.vector.tensor_reduce(yall[:], ytmp[:], axis=mybir.AxisListType.X, op=OP.add)

                # -------- c = -(beta/rho) * s * qp * y
                call = work.tile([P, B_LOC, NCHUNK], f32, tag="call")
                nc.vector.tensor_tensor(call[:], s_sim[:], qp[:], op=OP.mult)
                nc.vector.scalar_tensor_tensor(call[:], yall[:], -beta / rho, call[:],
                                               op0=OP.mult, op1=OP.mult)
                call_b16 = work.tile([P, B_LOC, NCHUNK], bf16, tag="call_b16")
                nc.vector.tensor_copy(call_b16[:], call[:])

                # -------- g_att: M=4 packed matmuls + diag extract + att update
                gatt_ps = psmall.tile([B_LOC, B_LOC, D], f32, tag="gatt")
                for c in range(NCHUNK):
                    nc.tensor.matmul(gatt_ps[:, :, :],
                                     call_b16[:, :, c],
                                     dpow[:, :, c, :],
                                     start=(c == 0), stop=(c == NCHUNK - 1))
                gm = work.tile([B_LOC, B_LOC, D], bf16, tag="gm")
                nc.vector.tensor_tensor(gm[:], gatt_ps[:], eye_bc, op=OP.mult)
                grow_ps = psum.tile([P, B_LOC, D], f32, tag="grow")
                nc.tensor.matmul(grow_ps[:, :, :].rearrange("p b d -> p (b d)"),
                                 ones4[:], gm[:].rearrange("p b d -> p (b d)"),
                                 start=True, stop=True)
                nc.vector.scalar_tensor_tensor(attb[:], grow_ps[:], -lr_att, attb[:],
                                               op0=OP.mult, op1=OP.add)
                nc.vector.tensor_scalar_max(attb[:], attb[:], 0.0)

                # -------- assoc update: assoc += (-lr*s_bc) * dx_bu (2 big TTs)
                slr = work.tile([P, B_LOC, NCHUNK], bf16, tag="slr")
                nc.vector.tensor_scalar_mul(slr[:], s_b16[:], -lr_assoc)
                upd = work.tile([P, B_LOC, NCHUNK, U], bf16, tag="upd", bufs=3)
                slr_bc = slr[:, :, :, None].broadcast_to([P, B_LOC, NCHUNK, U])
                nc.vector.tensor_tensor(upd[:], slr_bc, dxb_bc, op=OP.mult)
                nc.vector.tensor_tensor(assoc[:], assoc[:], upd[:], op=OP.add)

            # -------- store: (1, T, B, U) row -> (B, T*U), one DMA per batch
            for b in range(B_LOC):
                nc.gpsimd.dma_start(out_ext[b : b + 1, :].rearrange("b (t u) -> b t u", t=T),
                                    probs_row[0:1, :, b, :])

    nc.compile()
    return nc


def _prep_in_maps(stimulus_set, label_idx, embed):
    embedB = embed.reshape(NCHUNK, P, D).transpose(1, 0, 2).reshape(P, NCHUNK * D)
    z = embed[stimulus_set]  # (B, T, D)
    onehot = np.zeros((B, T, U), dtype=np.float32)
    bi, ti = np.meshgrid(np.arange(B), np.arange(T), indexing="ij")
    onehot[bi, ti, label_idx] = 1.0
    in_maps = []
    for i in range(N_CORES):
        bs = slice(i * B_LOC, (i + 1) * B_LOC)
        zc = z[bs].transpose(1, 0, 2).reshape(1, T * B_LOC * D)
        zbcast = np.broadcast_to(zc, (P, T * B_LOC * D))
        ohrow = onehot[bs].transpose(1, 0, 2).reshape(1, T * B_LOC * U)
        ohfull = np.broadcast_to(ohrow, (P, T * B_LOC * U))
        eyefull = np.zeros((P, B_LOC), dtype=np.float32)
        eyefull[:B_LOC, :] = np.eye(B_LOC, dtype=np.float32)
        import ml_dtypes
        bigb = np.concatenate([embedB, zbcast], axis=1).astype(ml_dtypes.bfloat16)
        big = np.concatenate([ohfull, eyefull], axis=1).astype(np.float32)
        in_maps.append({"bigb": np.ascontiguousarray(bigb),
                        "big": np.ascontiguousarray(big)})
    return in_maps


def kernel(stimulus_set, label_idx, embed, rho, temperature, lr_attention, lr_association, beta):
    from concourse.bass_utils import run_bass_kernel_spmd

    stimulus_set = np.asarray(stimulus_set)
    label_idx = np.asarray(label_idx)
    embed = np.asarray(embed, dtype=np.float32)
    key = (float(rho), float(temperature), float(lr_attention),
           float(lr_association), float(beta))
    if key not in _cache:
        _cache[key] = _build(*key)
    nc = _cache[key]
    in_maps = _prep_in_maps(stimulus_set, label_idx, embed)
    res = run_bass_kernel_spmd(nc, in_maps, core_ids=list(range(N_CORES)))
    outs = [res.results[i]["out"].reshape(B_LOC, T, U) for i in range(N_CORES)]
    return np.concatenate(outs, axis=0)


def _install_ntff_hook():
    import sys, types, ctypes, contextlib
    if "antenv.axon_hooks" in sys.modules:
        return
    import antenv
    mod = types.ModuleType("antenv.axon_hooks")
    mod._hook = None
    def set_axon_ntff_profile_hook(h):
        mod._hook = h
    def get_axon_ntff_profile_hook():
        return mod._hook
    mod.set_axon_ntff_profile_hook = set_axon_ntff_profile_hook
    mod.get_axon_ntff_profile_hook = get_axon_ntff_profile_hook
    sys.modules["antenv.axon_hooks"] = mod
    antenv.axon_hooks = mod

    lib = ctypes.CDLL("/opt/axon/libaxon_pjrt.so")
    if not hasattr(lib, "axon_start_nrt_profile"):
        return
    lib.axon_start_nrt_profile.argtypes = [ctypes.POINTER(ctypes.c_int64), ctypes.c_size_t]
    lib.axon_start_nrt_profile.restype = ctypes.c_int64
    lib.axon_stop_nrt_profile.argtypes = [ctypes.c_char_p]
    lib.axon_stop_nrt_profile.restype = ctypes.c_int64

    @contextlib.contextmanager
    def _hook(output_dir, device_ids):
        import jax
        jax.devices()
        if device_ids:
            ids = (ctypes.c_int64 * len(device_ids))(*device_ids)
            rc = lib.axon_start_nrt_profile(ids, len(device_ids))
        else:
            rc = lib.axon_start_nrt_profile(None, 0)
        if rc != 0:
            raise RuntimeError(f"axon_start_nrt_profile rc={rc}")
        try:
            yield
        finally:
            n = lib.axon_stop_nrt_profile(str(output_dir).encode())
            print(f"profile: {n} file(s) written to {output_dir}")

    set_axon_ntff_profile_hook(_hook)


def kernel_traced(**inputs):
    """Like kernel() but runs with NTFF tracing; returns (out, exec_time_ns, tmpdir)."""
    import tempfile
    _install_ntff_hook()
    from concourse.bass_utils import run_bass_kernel_spmd

    key = (float(inputs["rho"]), float(inputs["temperature"]), float(inputs["lr_attention"]),
           float(inputs["lr_association"]), float(inputs["beta"]))
    if key not in _cache:
        _cache[key] = _build(*key)
    nc = _cache[key]
    in_maps = _prep_in_maps(np.asarray(inputs["stimulus_set"]), np.asarray(inputs["label_idx"]),
                            np.asarray(inputs["embed"], dtype=np.float32))
    tmpdir = tempfile.mkdtemp(prefix="alcove_trace_")
    res = run_bass_kernel_spmd(nc, in_maps, core_ids=list(range(N_CORES)), trace=True, tmpdir=tmpdir)
    outs = [res.results[i]["out"].reshape(B_LOC, T, U) for i in range(N_CORES)]
    return np.concatenate(outs, axis=0), res.exec_time_ns, tmpdir


# revision 37
# speedup vs baseline: 1.3853x; 1.0893x over previous
"""ALCOVE cell Bass kernel for 8 TRN2 NeuronCores (data-parallel over batch).

B=32, T=16, N_RBF=1024, N_DIM=64, UNITS=64. 4 batches per core.

Layout: R=1024 on partitions as 8 chunks of 128; per-batch row data
(att, x, dx, g_att) on partition 0 as (1, B_LOC*64) rows (PE base-partition
rule); partition broadcasts via K=1 ones-matmul. Big elementwise work runs
on (128, B_LOC*NCHUNK*64) = (128, 2048) tiles in single instructions;
contractions over the free dim use TT-mult + tensor_reduce; contractions
over partitions use M=1 accumulating matmuls.
"""

import numpy as np

B, T, R, D, U = 32, 16, 1024, 64, 64
NCHUNK, P = 8, 128
EPS = 1e-6
N_CORES = 8
B_LOC = B // N_CORES  # 4

_cache = {}


def _patch_act_tables():
    """Make every activation resolve to natural_log_exp_and_others (it
    contains abs/ln/exp/relu/copy/identity/square) so the kernel needs a
    single ACT table load instead of thrashing between sets."""
    import concourse.bacc as bacc_mod
    from concourse.hw_specs import get_activation_tables as _gat

    if getattr(bacc_mod.get_activation_tables, "_alcove_patched", False):
        return

    def patched(arch):
        t = _gat(arch)
        keep = t["natural_log_exp_and_others"]
        out = {}
        for name, fns in t.items():
            out[name] = fns if name == "natural_log_exp_and_others" else (fns - keep)
        return out

    patched._alcove_patched = True
    bacc_mod.get_activation_tables = patched


def _build(rho, temperature, lr_att, lr_assoc, beta):
    import concourse.bass as bass
    import concourse.tile as tile
    from concourse import bacc, mybir

    _patch_act_tables()

    f32 = mybir.dt.float32
    bf16 = mybir.dt.bfloat16
    AF = mybir.ActivationFunctionType
    OP = mybir.AluOpType

    nc = bacc.Bacc("TRN2", target_bir_lowering=False, debug=False, num_devices=N_CORES)
    # packed bf16 input: [embedB (512) | zbcast (4096)]; f32 input: [oh (1024) | eye4 (4)]
    FINB = NCHUNK * D + T * B_LOC * D
    FIN = T * B_LOC * U + B_LOC
    bigb_in = nc.declare_dram_parameter("bigb", [P, FINB], bf16, isOutput=False)
    big_in = nc.declare_dram_parameter("big", [P, FIN], f32, isOutput=False)
    out_ext = nc.declare_dram_parameter("out", [B_LOC, T * U], f32, isOutput=True)

    with tile.TileContext(nc) as tc:
        with (
            tc.tile_pool(name="persist", bufs=1) as persist,
            tc.tile_pool(name="work", bufs=3) as work,
            tc.tile_pool(name="psum", bufs=1, space="PSUM") as psum,
            tc.tile_pool(name="psmall", bufs=2, space="PSUM") as psmall,
        ):
            # ---- persistent tiles (one DMA for all inputs) ----
            bigb = persist.tile([P, FINB], bf16)
            nc.gpsimd.dma_start(bigb[:], bigb_in[:])
            big = persist.tile([P, FIN], f32)
            nc.gpsimd.dma_start(big[:], big_in[:])
            embedB = bigb[:, 0 : NCHUNK * D]
            zb = bigb[:, NCHUNK * D :].rearrange("p (t f) -> p t f", t=T)
            # oh rows replicated on all partitions, layout (t, b, u)
            oh = big[0:B_LOC, 0 : T * B_LOC * U].rearrange("p (t b u) -> p t b u", t=T, b=B_LOC)
            eye4 = big[0:B_LOC, T * B_LOC * U :]  # (4, 4) identity
            eye_bc = eye4[:, :, None].broadcast_to([B_LOC, B_LOC, U])

            ones4 = persist.tile([B_LOC, P], bf16)
            nc.vector.memset(ones4[:], 1.0)
            consts = persist.tile([P, 3], f32)
            nc.vector.memset(consts[:, 0:1], 0.0)
            nc.vector.memset(consts[:, 1:2], 1.0)
            nc.vector.memset(consts[:, 2:3], EPS)
            czero, cone, ceps = consts[:, 0:1], consts[:, 1:2], consts[:, 2:3]

            attb = persist.tile([P, B_LOC, D], bf16)  # state, replicated on partitions
            nc.vector.memset(attb[:], 1.0 / D)
            assoc = persist.tile([P, B_LOC, NCHUNK, U], bf16)  # state
            nc.vector.memset(assoc[:], 0.0)

            probs_row = persist.tile([1, T, B_LOC, U], f32)

            # broadcast-view of embedB over batches: (P, B_LOC, NCHUNK, D)
            embed_bc = embedB.rearrange("p (c d) -> p c d", d=D)[:, None, :, :].broadcast_to([P, B_LOC, NCHUNK, D])

            for t in range(T):
                # -------- diff^rho chain on (P, B_LOC, NCHUNK, D) in one shot
                zrep = zb[:, t, :].rearrange("p (b d) -> p b d", d=D)[:, :, None, :].broadcast_to([P, B_LOC, NCHUNK, D])
                diff = work.tile([P, B_LOC, NCHUNK, D], bf16, tag="diff", bufs=4)
                nc.gpsimd.tensor_tensor(diff[:], embed_bc, zrep, op=OP.subtract)
                nc.scalar.activation(diff[:], diff[:], AF.Abs, bias=czero)
                nc.scalar.activation(diff[:], diff[:], AF.Ln, bias=ceps)
                dpow = work.tile([P, B_LOC, NCHUNK, D], bf16, tag="dpow", bufs=4)
                nc.scalar.activation(dpow[:], diff[:], AF.Exp, bias=czero, scale=rho)

                # -------- q = sum_d att*dpow : TT mult + reduce
                qtmp = work.tile([P, B_LOC, NCHUNK, D], bf16, tag="qtmp", bufs=3)
                attb_bc = attb[:, :, None, :].broadcast_to([P, B_LOC, NCHUNK, D])
                nc.vector.tensor_tensor(qtmp[:], dpow[:], attb_bc, op=OP.mult)
                qh = work.tile([P, B_LOC, NCHUNK, D // 2], bf16, tag="qh", bufs=2)
                nc.vector.tensor_tensor(qh[:], qtmp[:, :, :, 0 : D // 2],
                                        qtmp[:, :, :, D // 2 :], op=OP.add)
                qall = work.tile([P, B_LOC, NCHUNK], f32, tag="qall")
                nc.vector.tensor_reduce(qall[:], qh[:], axis=mybir.AxisListType.X, op=OP.add)

                # -------- similarity acts on (P, B_LOC*NCHUNK)
                lnq = work.tile([P, B_LOC, NCHUNK], f32, tag="lnq")
                nc.scalar.activation(lnq[:], qall[:], AF.Ln, bias=ceps)
                s_sim = work.tile([P, B_LOC, NCHUNK], f32, tag="s_sim")
                nc.scalar.activation(s_sim[:], lnq[:], AF.Exp, bias=czero, scale=1.0 / rho)  # = d
                nc.scalar.activation(s_sim[:], s_sim[:], AF.Exp, bias=czero, scale=-beta)    # = s
                qp = work.tile([P, B_LOC, NCHUNK], f32, tag="qp")
                nc.scalar.activation(qp[:], lnq[:], AF.Exp, bias=czero, scale=(1.0 - rho) / rho)
                s_b16 = work.tile([P, B_LOC, NCHUNK], bf16, tag="s_b16")
                nc.scalar.copy(s_b16[:], s_sim[:])

                # -------- x: M=4 packed matmuls (one per chunk), diag extract
                x_ps = psmall.tile([B_LOC, B_LOC, U], f32, tag="x_ps")
                for c in range(NCHUNK):
                    nc.tensor.matmul(x_ps[:, :, :],
                                     s_b16[:, :, c],
                                     assoc[:, :, c, :],
                                     start=(c == 0), stop=(c == NCHUNK - 1))
                # -------- teacher / dx on the cross tile (diag cols are the real x)
                pp = work.tile([B_LOC, B_LOC, U], f32, tag="pp")
                nc.scalar.activation(pp[:], x_ps[:], AF.Relu, bias=cone[:B_LOC, :])          # relu(x+1)
                mrow = work.tile([B_LOC, B_LOC, U], f32, tag="mrow")
                nc.scalar.activation(mrow[:], x_ps[:], AF.Relu, bias=cone[:B_LOC, :], scale=-1.0)  # relu(1-x)
                nc.vector.tensor_tensor(mrow[:], pp[:], mrow[:], op=OP.add)            # p+m
                nc.vector.tensor_tensor(mrow[:], mrow[:], oh[:, t, :, :], op=OP.mult)  # oh*(p+m)
                dxf = work.tile([B_LOC, B_LOC, U], f32, tag="dxf")
                nc.vector.tensor_tensor(dxf[:], pp[:], mrow[:], op=OP.subtract)
                # mask off-diagonal cross terms (bf16 out for single-pass matmuls)
                dxc = work.tile([B_LOC, B_LOC, U], bf16, tag="dxc")
                nc.vector.tensor_tensor(dxc[:], dxf[:], eye_bc, op=OP.mult)
                xm = work.tile([B_LOC, B_LOC, U], bf16, tag="xm")
                nc.vector.tensor_tensor(xm[:], x_ps[:], eye_bc, op=OP.mult)

                # -------- dx broadcast to (P, B_LOC, U): ones4^T @ dxm
                dxb_ps = psum.tile([P, B_LOC, U], f32, tag="dxb")
                nc.tensor.matmul(dxb_ps[:, :, :].rearrange("p b d -> p (b d)"),
                                 ones4[:], dxc[:].rearrange("p b u -> p (b u)"),
                                 start=True, stop=True)
                # x as a single row (1, B*U): ones4[:, :1]^T @ xm
                xrow_ps = psmall.tile([1, B_LOC, U], f32, tag="rowps")
                nc.tensor.matmul(xrow_ps[:, :, :].rearrange("p b u -> p (b u)"),
                                 ones4[:, 0:1], xm[:].rearrange("p b u -> p (b u)"),
                                 start=True, stop=True)

                # -------- softmax(temp*x) in row layout, per-b stable
                xr = work.tile([1, B_LOC, U], f32, tag="xr")
                nc.scalar.copy(xr[:], xrow_ps[:])
                mx4 = work.tile([1, B_LOC], f32, tag="mx4")
                nc.vector.tensor_reduce(mx4[:], xr[:], axis=mybir.AxisListType.X, op=OP.max)
                mx_bc = mx4[:, :, None].broadcast_to([1, B_LOC, U])
                xs_t = work.tile([1, B_LOC, U], f32, tag="xs_t")
                nc.gpsimd.tensor_tensor(xs_t[:], xr[:], mx_bc, op=OP.subtract)
                er = work.tile([1, B_LOC, U], f32, tag="er")
                nc.scalar.activation(er[:], xs_t[:], AF.Exp, bias=czero[:1, :], scale=temperature)
                sm4 = work.tile([1, B_LOC], f32, tag="sm4")
                nc.vector.tensor_reduce(sm4[:], er[:], axis=mybir.AxisListType.X, op=OP.add)
                rc4 = work.tile([1, B_LOC], f32, tag="rc4")
                nc.vector.reciprocal(rc4[:], sm4[:])
                rc_bc = rc4[:, :, None].broadcast_to([1, B_LOC, U])
                nc.gpsimd.tensor_tensor(probs_row[:, t, :, :], er[:], rc_bc, op=OP.mult)
                dxb = work.tile([P, B_LOC, U], bf16, tag="dxb_sb")
                nc.scalar.copy(dxb[:], dxb_ps[:])
                dxb_bc = dxb[:, :, None, :].broadcast_to([P, B_LOC, NCHUNK, U])

                # -------- y = sum_u assoc*dx : TT mult + reduce
                ytmp = work.tile([P, B_LOC, NCHUNK, U], bf16, tag="ytmp", bufs=3)
                nc.vector.tensor_tensor(ytmp[:], assoc[:], dxb_bc, op=OP.mult)
                yh = work.tile([P, B_LOC, NCHUNK, U // 2], bf16, tag="yh", bufs=2)
                nc.vector.tensor_tensor(yh[:], ytmp[:, :, :, 0 : U // 2],
                                        ytmp[:, :, :, U // 2 :], op=OP.add)
                yall = work.tile([P, B_LOC, NCHUNK], f32, tag="yall")
                nc.vector.tensor_reduce(yall[:], yh[:], axis=mybir.AxisListType.X, op=OP.add)

                # -------- c = -(beta/rho) * s * qp * y
                call = work.tile([P, B_LOC, NCHUNK], f32, tag="call")
                nc.vector.tensor_tensor(call[:], s_sim[:], qp[:], op=OP.mult)
                nc.vector.scalar_tensor_tensor(call[:], yall[:], -beta / rho, call[:],
                                               op0=OP.mult, op1=OP.mult)
                call_b16 = work.tile([P, B_LOC, NCHUNK], bf16, tag="call_b16")
                nc.scalar.copy(call_b16[:], call[:])

                # -------- g_att: M=4 packed matmuls + diag extract + att update
                gatt_ps = psmall.tile([B_LOC, B_LOC, D], f32, tag="gatt")
                for c in range(NCHUNK):
                    nc.tensor.matmul(gatt_ps[:, :, :],
                                     call_b16[:, :, c],
                                     dpow[:, :, c, :],
                                     start=(c == 0), stop=(c == NCHUNK - 1))
                gm = work.tile([B_LOC, B_LOC, D], bf16, tag="gm")
                nc.vector.tensor_tensor(gm[:], gatt_ps[:], eye_bc, op=OP.mult)
                grow_ps = psum.tile([P, B_LOC, D], f32, tag="grow")
                nc.tensor.matmul(grow_ps[:, :, :].rearrange("p b d -> p (b d)"),
                                 ones4[:], gm[:].rearrange("p b d -> p (b d)"),
                                 start=True, stop=True)
                nc.vector.scalar_tensor_tensor(attb[:], grow_ps[:], -lr_att, attb[:],
                                               op0=OP.mult, op1=OP.add)
                nc.vector.tensor_scalar_max(attb[:], attb[:], 0.0)

                # -------- assoc update: assoc += (-lr*s_bc) * dx_bu (2 big TTs)
                slr = work.tile([P, B_LOC, NCHUNK], bf16, tag="slr")
                nc.vector.tensor_scalar_mul(slr[:], s_b16[:], -lr_assoc)
                upd = work.tile([P, B_LOC, NCHUNK, U], bf16, tag="upd", bufs=3)
                slr_bc = slr[:, :, :, None].broadcast_to([P, B_LOC, NCHUNK, U])
                nc.vector.tensor_tensor(upd[:], dxb_bc, slr_bc, op=OP.mult)
                nc.vector.tensor_tensor(assoc[:], assoc[:], upd[:], op=OP.add)

            # -------- store: (1, T, B, U) row -> (B, T*U), one DMA per batch
            for b in range(B_LOC):
                nc.gpsimd.dma_start(out_ext[b : b + 1, :].rearrange("b (t u) -> b t u", t=T),
                                    probs_row[0:1, :, b, :])

    nc.compile()
    return nc


def _prep_in_maps(stimulus_set, label_idx, embed):
    embedB = embed.reshape(NCHUNK, P, D).transpose(1, 0, 2).reshape(P, NCHUNK * D)
    z = embed[stimulus_set]  # (B, T, D)
    onehot = np.zeros((B, T, U), dtype=np.float32)
    bi, ti = np.meshgrid(np.arange(B), np.arange(T), indexing="ij")
    onehot[bi, ti, label_idx] = 1.0
    in_maps = []
    for i in range(N_CORES):
        bs = slice(i * B_LOC, (i + 1) * B_LOC)
        zc = z[bs].transpose(1, 0, 2).reshape(1, T * B_LOC * D)
        zbcast = np.broadcast_to(zc, (P, T * B_LOC * D))
        ohrow = onehot[bs].transpose(1, 0, 2).reshape(1, T * B_LOC * U)
        ohfull = np.broadcast_to(ohrow, (P, T * B_LOC * U))
        eyefull = np.zeros((P, B_LOC), dtype=np.float32)
        eyefull[:B_LOC, :] = np.eye(B_LOC, dtype=np.float32)
        import ml_dtypes
        bigb = np.concatenate([embedB, zbcast], axis=1).astype(ml_dtypes.bfloat16)
        big = np.concatenate([ohfull, eyefull], axis=1).astype(np.float32)
        in_maps.append({"bigb": np.ascontiguousarray(bigb),
                        "big": np.ascontiguousarray(big)})
    return in_maps


def kernel(stimulus_set, label_idx, embed, rho, temperature, lr_attention, lr_association, beta):
    from concourse.bass_utils import run_bass_kernel_spmd

    stimulus_set = np.asarray(stimulus_set)
    label_idx = np.asarray(label_idx)
    embed = np.asarray(embed, dtype=np.float32)
    key = (float(rho), float(temperature), float(lr_attention),
           float(lr_association), float(beta))
    if key not in _cache:
        _cache[key] = _build(*key)
    nc = _cache[key]
    in_maps = _prep_in_maps(stimulus_set, label_idx, embed)
    res = run_bass_kernel_spmd(nc, in_maps, core_ids=list(range(N_CORES)))
    outs = [res.results[i]["out"].reshape(B_LOC, T, U) for i in range(N_CORES)]
    return np.concatenate(outs, axis=0)


def _install_ntff_hook():
    import sys, types, ctypes, contextlib
    if "antenv.axon_hooks" in sys.modules:
        return
    import antenv
    mod = types.ModuleType("antenv.axon_hooks")
    mod._hook = None
    def set_axon_ntff_profile_hook(h):
        mod._hook = h
    def get_axon_ntff_profile_hook():
        return mod._hook
    mod.set_axon_ntff_profile_hook = set_axon_ntff_profile_hook
    mod.get_axon_ntff_profile_hook = get_axon_ntff_profile_hook
    sys.modules["antenv.axon_hooks"] = mod
    antenv.axon_hooks = mod

    lib = ctypes.CDLL("/opt/axon/libaxon_pjrt.so")
    if not hasattr(lib, "axon_start_nrt_profile"):
        return
    lib.axon_start_nrt_profile.argtypes = [ctypes.POINTER(ctypes.c_int64), ctypes.c_size_t]
    lib.axon_start_nrt_profile.restype = ctypes.c_int64
    lib.axon_stop_nrt_profile.argtypes = [ctypes.c_char_p]
    lib.axon_stop_nrt_profile.restype = ctypes.c_int64

    @contextlib.contextmanager
    def _hook(output_dir, device_ids):
        import jax
        jax.devices()
        if device_ids:
            ids = (ctypes.c_int64 * len(device_ids))(*device_ids)
            rc = lib.axon_start_nrt_profile(ids, len(device_ids))
        else:
            rc = lib.axon_start_nrt_profile(None, 0)
        if rc != 0:
            raise RuntimeError(f"axon_start_nrt_profile rc={rc}")
        try:
            yield
        finally:
            n = lib.axon_stop_nrt_profile(str(output_dir).encode())
            print(f"profile: {n} file(s) written to {output_dir}")

    set_axon_ntff_profile_hook(_hook)


def kernel_traced(**inputs):
    """Like kernel() but runs with NTFF tracing; returns (out, exec_time_ns, tmpdir)."""
    import tempfile
    _install_ntff_hook()
    from concourse.bass_utils import run_bass_kernel_spmd

    key = (float(inputs["rho"]), float(inputs["temperature"]), float(inputs["lr_attention"]),
           float(inputs["lr_association"]), float(inputs["beta"]))
    if key not in _cache:
        _cache[key] = _build(*key)
    nc = _cache[key]
    in_maps = _prep_in_maps(np.asarray(inputs["stimulus_set"]), np.asarray(inputs["label_idx"]),
                            np.asarray(inputs["embed"], dtype=np.float32))
    tmpdir = tempfile.mkdtemp(prefix="alcove_trace_")
    res = run_bass_kernel_spmd(nc, in_maps, core_ids=list(range(N_CORES)), trace=True, tmpdir=tmpdir)
    outs = [res.results[i]["out"].reshape(B_LOC, T, U) for i in range(N_CORES)]
    return np.concatenate(outs, axis=0), res.exec_time_ns, tmpdir
